# revision 2
# baseline (speedup 1.0000x reference)
"""AdMSoftmaxLoss distributed Trainium2 kernel.

Reference computation (N=8192, D=1024, C=10240, S=30, ml=0.4, ms=0.1):
    wf    = clip(l2norm(x) @ l2norm(weight).T, -1, 1)      # (N, C) cosines
    m     = where(labels <= 5, ml, ms)
    t     = wf[i, labels[i]]
    num   = S * (t - m)
    excl  = sum_j exp(S * wf[i, j]) - exp(S * t)
    L     = num - log(exp(num) + excl)
    loss  = -mean(L)

Sharding: 2 row-groups x 4 class-groups over 8 NeuronCores. Core i gets
rows [ (i//4)*4096, .. ) and classes [ (i%4)*2560, .. ). Each core
computes, for its (row, class) block:
    out[0][r] = sum_{c in block} exp(S * cos[r, c])       (partial denom)
    out[1][r] = exp(S * cos[r, labels[r]]) if label owned  (partial)
The host sums partials over class groups, recovers t = log(out1)/S, and
finishes the O(N) loss arithmetic (one million times less work than the
device-side matmul).

Device pipeline per core:
  - weight: SWDGE dma cast f32->bf16, Square+accum row norms (ScalarE),
    reciprocal+sqrt -> 1/||w||, scale rows to unit norm (bf16), write to
    DRAM scratch, then 8 big DMA-xbar transposes -> wnT (d-major).
  - x: SWDGE cast to bf16 per 128-row tile, Square+accum norms; the
    1/||x|| factor is folded into the ScalarE exp as a per-partition
    activation scale (30/||x||), so x is NOT scaled: matmul computes
    dot(x_bf16, wn_bf16) = cos * ||x||.
  - matmul: 32 m-tiles x 5 n-chunks(512) x 8 k-tiles, bf16, PSUM f32.
  - epilogue per chunk: ScalarE activation Exp(scale=30/||x||) with
    accum_out (fused row-sum); VectorE scalar_tensor_tensor
    (iota == label-offset) * exp with accum_out (fused label gather).
"""

import os
import numpy as np

P = 128
N_ROWS, D, C = 8192, 1024, 10240
S = 30.0
ML, MS = 0.4, 0.1
NCORES = 8
RG, CG = 2, 4                  # row groups x class groups
R_LOC = N_ROWS // RG           # 4096
C_LOC = C // CG                # 2560
M_TILES = R_LOC // P           # 32
NCHUNK = 512
N_CHUNKS = C_LOC // NCHUNK     # 5
K_TILES = D // P               # 8
W_TILES = C_LOC // P           # 20
GROUPS = 4                     # x prep/transpose pipeline groups
G_MT = M_TILES // GROUPS       # 8 m-tiles per group
G_ROWS = R_LOC // GROUPS       # 1024

_CACHE = {}
LAST_RESULTS = None  # BassKernelResults of the most recent run (for test.py)


def _build():
    """Build + compile the SPMD Bass graph once; cache in module global."""
    if "nc" in _CACHE:
        return _CACHE["nc"]

    import concourse.bass as bass
    import concourse.mybir as mybir
    import concourse.tile as tile
    from concourse import bacc

    ts = bass.ts
    dt = mybir.dt
    AF = mybir.ActivationFunctionType
    ALU = mybir.AluOpType

    nc = bacc.Bacc(
        "TRN2", target_bir_lowering=False, debug=False, num_devices=NCORES
    )

    x_ext = nc.dram_tensor("x", [R_LOC, D], dt.float32, kind="ExternalInput").ap()
    w_ext = nc.dram_tensor("w", [C_LOC, D], dt.float32, kind="ExternalInput").ap()
    lab_ext = nc.dram_tensor(
        "lab", [P, M_TILES], dt.float32, kind="ExternalInput"
    ).ap()
    iota_ext = nc.dram_tensor(
        "iota", [P, NCHUNK], dt.float32, kind="ExternalInput"
    ).ap()
    noff_ext = nc.dram_tensor(
        "noff", [P, N_CHUNKS], dt.float32, kind="ExternalInput"
    ).ap()
    out_ext = nc.dram_tensor(
        "out", [2, P, M_TILES], dt.float32, kind="ExternalOutput"
    ).ap()

    with tile.TileContext(nc) as tc:
        with (
            tc.tile_pool(name="dram", bufs=1, space="DRAM") as dram,
            tc.tile_pool(name="consts", bufs=1) as consts,
            tc.tile_pool(name="wstage", bufs=3) as wstage,
            tc.tile_pool(name="xstage", bufs=3) as xstage,
            tc.tile_pool(name="sq", bufs=2) as sqpool,
            tc.tile_pool(name="small", bufs=4) as small,
            tc.tile_pool(name="xnt", bufs=2) as xnt_pool,
            tc.tile_pool(name="epi", bufs=3) as epi,
            tc.tile_pool(name="psum", bufs=8, space="PSUM") as psum,
        ):
            xb_dram = dram.tile([R_LOC, D], dt.bfloat16)
            wb_dram = dram.tile([C_LOC, D], dt.bfloat16)

            iota_sb = consts.tile([P, NCHUNK], dt.float32)
            nc.sync.dma_start(iota_sb[:], iota_ext)
            noff_sb = consts.tile([P, N_CHUNKS], dt.float32)
            nc.sync.dma_start(noff_sb[:], noff_ext)
            lab_sb = consts.tile([P, M_TILES], dt.float32)
            nc.sync.dma_start(lab_sb[:], lab_ext)

            outsum = consts.tile([P, M_TILES], dt.float32)
            outtgt = consts.tile([P, M_TILES], dt.float32)

            # wnT[d_partition, k, class] : d-major normalized weight, bf16
            wnT = consts.tile([P, K_TILES, C_LOC], dt.bfloat16)

            # ---- weight prep ----
            for wt in range(W_TILES):
                wtile = wstage.tile([P, D], dt.bfloat16, tag="wtile")
                nc.gpsimd.dma_start(wtile[:], w_ext[ts(wt, P), :])  # cast f32->bf16
                sq = sqpool.tile([P, D], dt.float32, tag="sq")
                ns = small.tile([P, 1], dt.float32, tag="wns")
                nc.scalar.activation(sq[:], wtile[:], AF.Square, accum_out=ns[:])
                rns = small.tile([P, 1], dt.float32, tag="wrns")
                nc.vector.reciprocal(rns[:], ns[:])
                winv = small.tile([P, 1], dt.float32, tag="winv")
                nc.scalar.activation(winv[:], rns[:], AF.Sqrt)  # 1/||w||
                wn = wstage.tile([P, D], dt.bfloat16, tag="wn")
                nc.vector.tensor_scalar_mul(wn[:], wtile[:], winv[:])
                nc.sync.dma_start(wb_dram[ts(wt, P), :], wn[:])
            for k in range(K_TILES):
                nc.sync.dma_start_transpose(wnT[:, k, :], wb_dram[:, ts(k, P)])

            # ---- x prep + matmul + epilogue, pipelined in row groups ----
            for g in range(GROUPS):
                scl30 = small.tile([P, G_MT], dt.float32, tag="scl30")
                for j in range(G_MT):
                    m = g * G_MT + j
                    xt = xstage.tile([P, D], dt.bfloat16, tag="xt")
                    nc.gpsimd.dma_start(xt[:], x_ext[ts(m, P), :])  # cast
                    sqx = sqpool.tile([P, D], dt.float32, tag="sq")
                    nsx = small.tile([P, 1], dt.float32, tag="xns")
                    nc.scalar.activation(sqx[:], xt[:], AF.Square, accum_out=nsx[:])
                    rx = small.tile([P, 1], dt.float32, tag="xrns")
                    nc.vector.reciprocal(rx[:], nsx[:])
                    # sqrt(900/ns) = 30/||x||  (exp scale, folded normalize)
                    nc.scalar.activation(
                        scl30[:, j : j + 1], rx[:], AF.Sqrt, scale=S * S
                    )
                    nc.sync.dma_start(xb_dram[ts(m, P), :], xt[:])
                xnT = xnt_pool.tile([P, K_TILES, G_ROWS], dt.bfloat16, tag="xnT")
                for k in range(K_TILES):
                    nc.sync.dma_start_transpose(
                        xnT[:, k, :], xb_dram[ts(g, G_ROWS), ts(k, P)]
                    )

                for j in range(G_MT):
                    m = g * G_MT + j
                    labadj = small.tile([P, N_CHUNKS], dt.float32, tag="labadj")
                    nc.vector.tensor_scalar(
                        labadj[:], noff_sb[:], lab_sb[:, m : m + 1], None, ALU.add
                    )
                    sums = small.tile([P, N_CHUNKS], dt.float32, tag="sums")
                    tgts = small.tile([P, N_CHUNKS], dt.float32, tag="tgts")
                    for n in range(N_CHUNKS):
                        ps = psum.tile([P, NCHUNK], dt.float32, tag="ps")
                        for k in range(K_TILES):
                            nc.tensor.matmul(
                                ps[:],
                                xnT[:, k, ts(j, P)],
                                wnT[:, k, ts(n, NCHUNK)],
                                start=(k == 0),
                                stop=(k == K_TILES - 1),
                            )
                        esc = epi.tile([P, NCHUNK], dt.float32, tag="esc")
                        nc.scalar.activation(
                            esc[:],
                            ps[:],
                            AF.Exp,
                            scale=scl30[:, j : j + 1],
                            accum_out=sums[:, n : n + 1],
                        )
                        msc = epi.tile([P, NCHUNK], dt.float32, tag="msc")
                        nc.vector.scalar_tensor_tensor(
                            msc[:],
                            iota_sb[:],
                            labadj[:, n : n + 1],
                            esc[:],
                            op0=ALU.is_equal,
                            op1=ALU.mult,
                            accum_out=tgts[:, n : n + 1],
                        )
                    nc.vector.tensor_reduce(
                        outsum[:, m : m + 1],
                        sums[:],
                        axis=mybir.AxisListType.X,
                        op=ALU.add,
                    )
                    nc.vector.tensor_reduce(
                        outtgt[:, m : m + 1],
                        tgts[:],
                        axis=mybir.AxisListType.X,
                        op=ALU.add,
                    )

            nc.sync.dma_start(out_ext[0], outsum[:])
            nc.sync.dma_start(out_ext[1], outtgt[:])

    nc.compile()
    _CACHE["nc"] = nc
    return nc


def _make_in_maps(x, labels, weight):
    iota = np.broadcast_to(
        np.arange(NCHUNK, dtype=np.float32)[None, :], (P, NCHUNK)
    ).copy()
    noff = np.broadcast_to(
        (-NCHUNK * np.arange(N_CHUNKS, dtype=np.float32))[None, :], (P, N_CHUNKS)
    ).copy()
    labels_f = labels.astype(np.float32)
    in_maps = []
    for i in range(NCORES):
        gr, ci = divmod(i, CG)
        xs = np.ascontiguousarray(x[gr * R_LOC : (gr + 1) * R_LOC])
        ws = np.ascontiguousarray(weight[ci * C_LOC : (ci + 1) * C_LOC])
        lab = labels_f[gr * R_LOC : (gr + 1) * R_LOC] - ci * C_LOC
        lab_shuf = np.ascontiguousarray(lab.reshape(M_TILES, P).T)
        in_maps.append(
            {"x": xs, "w": ws, "lab": lab_shuf, "iota": iota, "noff": noff}
        )
    return in_maps


def kernel(x, labels, weight):
    global LAST_RESULTS
    from concourse.bass_utils import run_bass_kernel_spmd

    x = np.asarray(x, dtype=np.float32)
    weight = np.asarray(weight, dtype=np.float32)
    labels = np.asarray(labels)

    nc = _build()
    in_maps = _make_in_maps(x, labels, weight)
    trace = bool(int(os.environ.get("ADMS_TRACE", "0")))
    res = run_bass_kernel_spmd(
        nc, in_maps, list(range(NCORES)), trace=trace
    )
    LAST_RESULTS = res

    total = np.zeros(N_ROWS, np.float64)
    tgtexp = np.zeros(N_ROWS, np.float64)
    for i, r in enumerate(res.results):
        gr = i // CG
        o = np.asarray(r["out"], dtype=np.float64).reshape(2, P, M_TILES)
        part = o.transpose(0, 2, 1).reshape(2, R_LOC)  # [s, m*P + p]
        sl = slice(gr * R_LOC, (gr + 1) * R_LOC)
        total[sl] += part[0]
        tgtexp[sl] += part[1]

    t = np.log(tgtexp) / S
    t = np.clip(t, -1.0, 1.0)
    m = np.where(labels <= 5, ML, MS)
    num = S * (t - m)
    L = num - np.log(np.exp(num) + (total - tgtexp))
    return np.float32(-L.mean())
